# revision 24
# baseline (speedup 1.0000x reference)
"""Trainium2 Bass kernel for nn_CaduceusEmbeddingsSTFT.

out[b, t, :] = concat(emb_table[ids[b, t]],
                      proj(|STFT(onehot(ids[b]))| upsampled at frame f(t)))

Structure exploited:
  * nearest upsampling -> only 65 distinct STFT frame rows per core-half;
    the projection collapses to (65 x 2064) @ (2064 x 154).
  * STFT of one-hot signals: windowed frames are one-hot masks, so
    spec = onehot_frames @ (window * DFT) as matmuls (cos / sin).
  * every output row is concat(emb_row[id(t)], S[frame(t)]) -- built by a
    SINGLE matmul per 128-row tile: lhsT stacks the id one-hot (16 rows)
    and the frame one-hot (65 rows), rhs is the table [emb | S].

Precision: harness gate is rel_err < 2e-2; everything runs plain bf16
(fp32 PSUM accumulation), output DMA'd as bf16 and upcast on host.
Measured numpy sim of this scheme: rel err ~4.8e-3.

Sharding: 8 cores = 4 batches x 2 sequence halves; each core computes a
(4096, 512) output shard; boundary frame recomputed by both halves.

Perf design (per core): ~18 dummy matmuls pre-warm the PE HAM clock
while input DMAs land; DFT (6 streams x 1040 cols) -> |mag| on ACT/DVE
-> projection (16 K=128 MMs + 16 K=1 nyquist MMs, bias via DVE
broadcast-add into the rhs table) -> 32 fused N=512 output MMs; drains
alternate DVE/ACT; output leaves as bf16 (4 MB/core, ~12 us DMA).
"""

import numpy as np

V = 16
D_EMB = 358
D_STFT = 154
NFFT = 256
HOP = 64
NFREQ = 129
B, L = 4, 8192
LH = L // 2  # 4096 rows per core
F = 65  # frames per core (inclusive overlap frame)
VF = V * F  # 1040
DM = 512
NCORES = 8
NT = LH // 128  # 32 output tiles per core
NQ = NT // 4  # q-groups of 4 tiles
KOUT = V + F  # 81: stacked one-hot rows in the output matmul
CWW = 2 * NFREQ  # 258: per-c block width in cw (cos 0..127 | ny | sin 0..127)
# (start, size) chunks over the VF axis; multiples of F so projection
# lhsT slices [:, v*F:(v+1)*F] never cross a chunk boundary; <=512 f32
# per PSUM bank.
CHUNKS = [(0, 7 * F), (7 * F, 7 * F), (14 * F, 2 * F)]
NDUM = 15  # PE warm-up matmuls issued while the first input DMA lands
NFILL2 = 10  # keep-warm matmuls ready with chunk-2 |mag|
NFILL3 = 14  # keep-warm matmuls ready with chunk-3 |mag|

PK1W = 2 * CWW + 2 * VF  # [cw | c-interleaved ohf chunk blocks]
BFW = DM + LH  # [rhs-table init image | one-hot select cols]

_PROG = None
LAST_RESULT = None  # BassKernelResults of the most recent run (for harnesses)


def _build_program():
    import concourse.mybir as mybir
    import concourse.tile as tile
    from concourse import bacc

    f32 = mybir.dt.float32
    bf16 = mybir.dt.bfloat16
    AO = mybir.AluOpType
    AF = mybir.ActivationFunctionType

    nc = bacc.Bacc("TRN2", target_bir_lowering=False, debug=False,
                   num_devices=NCORES)

    pk1 = nc.dram_tensor("pk1", [128, PK1W], bf16, kind="ExternalInput")
    pk2 = nc.dram_tensor("pk2", [128, V * D_STFT], bf16, kind="ExternalInput")
    nyw = nc.dram_tensor("nyw", [1, V * D_STFT], bf16, kind="ExternalInput")
    bfsel = nc.dram_tensor("bfsel", [KOUT, BFW], bf16, kind="ExternalInput")
    out = nc.dram_tensor("out", [LH, DM], bf16, kind="ExternalOutput")

    with tile.TileContext(nc) as tc:
        with (
            tc.tile_pool(name="consts", bufs=1) as cpool,
            tc.tile_pool(name="work", bufs=1) as wpool,
            tc.tile_pool(name="tmp", bufs=2) as tpool,
            tc.tile_pool(name="ostg", bufs=3) as ospool,
        ):
            # ---- const loads, split so consumers start ASAP ----------------
            # issue order: nyw (tiny), cw+chunk blocks, wproj, bf halves
            NYW = cpool.tile([1, V * D_STFT], bf16, tag="nyw")
            nc.sync.dma_start(out=NYW[:], in_=nyw[:])
            PK1 = cpool.tile([128, PK1W], bf16, tag="pk1")
            CB = 2 * CWW  # chunk blocks start after cw
            nc.sync.dma_start(out=PK1[:, :CB + 2 * CHUNKS[0][1]],
                              in_=pk1[:, :CB + 2 * CHUNKS[0][1]])
            for c0, cn in CHUNKS[1:]:
                nc.sync.dma_start(out=PK1[:, CB + 2 * c0:CB + 2 * (c0 + cn)],
                                  in_=pk1[:, CB + 2 * c0:CB + 2 * (c0 + cn)])
            PK2 = cpool.tile([128, V * D_STFT], bf16, tag="pk2")
            nc.sync.dma_start(out=PK2[:], in_=pk2[:])
            BF = cpool.tile([KOUT, BFW], bf16, tag="bf")
            BH = DM + LH // 2
            nc.sync.dma_start(out=BF[:, :BH], in_=bfsel[:, :BH])
            nc.sync.dma_start(out=BF[:, BH:], in_=bfsel[:, BH:])

            CW = PK1[:, 0:CB]
            WP = PK2
            WNR = NYW

            def ohf_rhs(c, c0, cn):
                base = CB + 2 * c0 + c * cn
                return PK1[:, base:base + cn]

            def bf_lhsT(ti):
                return BF[:, DM + ti * 128:DM + (ti + 1) * 128]

            # ---- on-chip work tiles ----------------------------------------
            ZW = wpool.tile([128, 128], bf16, tag="zw")
            nc.vector.memset(ZW[:], 0.0)
            # rhs table init: rows 0..64 [0 | bias], rows 65..80 [emb | 0]
            RT = wpool.tile([KOUT, DM], bf16, tag="rt")
            nc.vector.tensor_copy(out=RT[:], in_=BF[:, :DM])
            MAGH = wpool.tile([128, VF], bf16, tag="magh")
            NYB = wpool.tile([1, VF], bf16, tag="nyb")

            with tc.tile_pool(name="psum_s", bufs=1, space="PSUM") as psp:
                S = psp.tile([F, D_STFT], f32, tag="s")

                with (
                    tc.tile_pool(name="psum_dum", bufs=1, space="PSUM") as pdm,
                    tc.tile_pool(name="psum_re", bufs=2, space="PSUM") as pre,
                    tc.tile_pool(name="psum_im", bufs=2, space="PSUM") as pim,
                    tc.tile_pool(name="psum_ny", bufs=2, space="PSUM") as pny,
                ):
                    # PE warm-up: no input deps, scheduler runs these first;
                    # ~18 x (ldw+mm) ~= 3.4us busy -> HAM releases to 2.4 GHz
                    # right as the first real matmul's data lands.
                    DU = pdm.tile([128, 128], f32, tag="du")
                    for _ in range(NDUM):
                        nc.tensor.matmul(out=DU[:], lhsT=ZW[:], rhs=ZW[:],
                                         start=True, stop=True)

                    first_s = [True]

                    def proj_mm(lhsT, rhs, stop=False):
                        nc.tensor.matmul(out=S[:], lhsT=lhsT, rhs=rhs,
                                         start=first_s[0], stop=stop)
                        first_s[0] = False

                    last = CHUNKS[-1]
                    for c0, cn in CHUNKS:
                        re = pre.tile([128, 7 * F], f32, tag="re")
                        im = pim.tile([128, 7 * F], f32, tag="im")
                        ny = pny.tile([1, 7 * F], f32, tag="ny")
                        for c in range(2):
                            cb = c * CWW
                            rhs = ohf_rhs(c, c0, cn)
                            nc.tensor.matmul(
                                out=re[:, :cn], lhsT=CW[:, cb:cb + 128],
                                rhs=rhs, start=(c == 0), stop=(c == 1))
                            nc.tensor.matmul(
                                out=im[:, :cn],
                                lhsT=CW[:, cb + NFREQ:cb + NFREQ + 128],
                                rhs=rhs, start=(c == 0), stop=(c == 1))
                            nc.tensor.matmul(
                                out=ny[:, :cn], lhsT=CW[:, cb + 128:cb + 129],
                                rhs=rhs, start=(c == 0), stop=(c == 1))
                        with tc.high_priority():
                            # |ny| first so the tiny ny matmuls unlock early
                            nc.scalar.activation(NYB[:, c0:c0 + cn],
                                                 ny[:, :cn], AF.Abs)
                            # |spec| = sqrt(re^2 + im^2); squares on ACT (DVE
                            # cannot read two PSUM operands)
                            t1 = tpool.tile([128, 7 * F], f32, tag="sq1")
                            t2 = tpool.tile([128, 7 * F], f32, tag="sq2")
                            nc.scalar.square(out=t1[:, :cn], in_=re[:, :cn])
                            nc.scalar.square(out=t2[:, :cn], in_=im[:, :cn])
                            nc.vector.tensor_tensor(
                                out=t1[:, :cn], in0=t1[:, :cn], in1=t2[:, :cn],
                                op=AO.add)
                            nc.scalar.sqrt(out=MAGH[:, c0:c0 + cn],
                                           in_=t1[:, :cn])
                            # nyquist bin: tiny K=1 matmuls, ready early --
                            # they keep the PE fed while ACT grinds the mags
                            for v in range(c0 // F, (c0 + cn) // F):
                                proj_mm(NYB[0:1, v * F:(v + 1) * F],
                                        WNR[0:1,
                                            v * D_STFT:(v + 1) * D_STFT])
                            for v in range(c0 // F, (c0 + cn) // F):
                                proj_mm(MAGH[:, v * F:(v + 1) * F],
                                        WP[:, v * D_STFT:(v + 1) * D_STFT],
                                        stop=((c0, cn) == last
                                              and v == (c0 + cn) // F - 1))

                    # keep-warm fillers: become ready with chunk-2/3 |mag|,
                    # so the scheduler slots them into PE idle spots while
                    # the S tail runs on ACT/DVE. Without these the ~3us gap
                    # re-throttles HAM and the output phase runs at 1.2 GHz.
                    for _ in range(NFILL2):
                        nc.tensor.matmul(out=DU[:],
                                         lhsT=MAGH[:, 14 * F - 128:14 * F],
                                         rhs=ZW[:], start=True, stop=True)
                    for _ in range(NFILL3):
                        nc.tensor.matmul(out=DU[:],
                                         lhsT=MAGH[:, VF - 128:VF],
                                         rhs=ZW[:], start=True, stop=True)

                with tc.high_priority():
                    # rhs table rows 0..64 = S + bias image (in-place add)
                    nc.vector.tensor_tensor(
                        out=RT[0:F, D_EMB:DM], in0=S[:],
                        in1=RT[0:F, D_EMB:DM], op=AO.add)

            # ---- output: one fused matmul per 128-row tile ------------------
            # MMs (~216ns) outpace the DVE/ACT drains (~330ns/tile): the PE
            # micro-idles once PSUM fills, which re-throttles HAM mid-phase.
            # Two fillers per q-group (gated on this group's drains) keep
            # its activity window busy.
            with (
                tc.tile_pool(name="psum_out", bufs=7, space="PSUM") as pout,
                tc.tile_pool(name="psum_fil", bufs=1, space="PSUM") as pfil,
            ):
                for q in range(NQ):
                    os_ = ospool.tile([128, 4 * DM], bf16, tag="os")
                    for a in range(4):
                        ti = q * 4 + a
                        po = pout.tile([128, DM], f32, tag="po")
                        nc.tensor.matmul(
                            out=po[:], lhsT=bf_lhsT(ti),
                            rhs=RT[0:KOUT, :], start=True, stop=True)
                        sl = os_[:, a * DM:(a + 1) * DM]
                        if a % 2 == 0:
                            nc.vector.tensor_copy(out=sl, in_=po[:])
                        else:
                            nc.scalar.copy(out=sl, in_=po[:])
                    DU2 = pfil.tile([128, 128], f32, tag="fil")
                    nc.tensor.matmul(out=DU2[:], lhsT=os_[:, 0:128],
                                     rhs=ZW[:], start=True, stop=True)
                    nc.tensor.matmul(out=DU2[:], lhsT=os_[:, DM:DM + 128],
                                     rhs=ZW[:], start=True, stop=True)
                    nc.sync.dma_start(
                        out=out[q * 512:(q + 1) * 512, :]
                        .rearrange("(a p) e -> p a e", p=128),
                        in_=os_[:].rearrange("p (a e) -> p a e", a=4))

    nc.finalize()
    return nc


def _host_consts():
    import ml_dtypes

    bf16 = ml_dtypes.bfloat16
    n = np.arange(NFFT)
    window = 0.5 - 0.5 * np.cos(2.0 * np.pi * n / NFFT)
    k = np.arange(NFREQ)
    ang = 2.0 * np.pi * np.outer(n, k) / NFFT  # (256, 129)
    wcos = (window[:, None] * np.cos(ang)).astype(np.float32)
    wsin = (window[:, None] * np.sin(ang)).astype(np.float32)
    cw = np.zeros((128, 2 * CWW), np.float32)
    for c in range(2):
        rows = slice(c * 128, (c + 1) * 128)
        blk = np.zeros((128, CWW), np.float32)
        blk[:, :128] = wcos[rows, :128]
        blk[:, 128] = wcos[rows][:, 128]  # nyquist cos column
        blk[:, NFREQ:NFREQ + 128] = wsin[rows, :128]
        cw[:, c * CWW:(c + 1) * CWW] = blk
    return cw.astype(bf16)


def kernel(input_ids, emb_table, proj_w, proj_b):
    global _PROG, LAST_RESULT
    import ml_dtypes

    from concourse.bass_utils import run_bass_kernel_spmd

    bf16 = ml_dtypes.bfloat16
    ids = np.asarray(input_ids).astype(np.int64)
    emb = np.asarray(emb_table).astype(np.float32)
    pw = np.asarray(proj_w).astype(np.float32)
    pb = np.asarray(proj_b).astype(np.float32)

    cw = _host_consts()

    # pk1 cols [2*VF:] = cw; per-core ohf fills cols [:2*VF]
    # pk2: proj weights, rows k=0..127, cols v*154+o  (proj_w row i=k*V+v)
    pk2 = np.zeros((128, V * D_STFT), np.float32)
    for v in range(V):
        pk2[:, v * D_STFT:(v + 1) * D_STFT] = pw[np.arange(128) * V + v]
    pk2 = pk2.astype(bf16)

    # nyw: nyquist-bin proj weights as a single partition-0 row
    nywr = np.zeros((1, V * D_STFT), np.float32)
    for v in range(V):
        nywr[0, v * D_STFT:(v + 1) * D_STFT] = pw[128 * V + v]
    nywr = nywr.astype(bf16)

    # rhs-table init image: rows 0..64 [0 | bias], rows 65..80 [emb | 0]
    rtimg = np.zeros((KOUT, DM), np.float32)
    rtimg[:F, D_EMB:] = pb[None, :]
    rtimg[F:, :D_EMB] = emb

    vr = np.arange(V)
    in_maps = []
    for core in range(NCORES):
        b, h = divmod(core, 2)
        padded = np.pad(ids[b], 128, mode="reflect")
        seg = padded[LH * h:LH * h + 64 * (F - 1) + NFFT]  # (4352,)
        pk1 = np.zeros((128, PK1W), bf16)
        pk1[:, :2 * CWW] = cw
        ohf = []
        for c in range(2):
            sv = seg[(128 * c + np.arange(128))[:, None]
                     + 64 * np.arange(F)[None, :]]  # (128, F)
            oh = (sv[:, None, :] == vr[None, :, None])  # (128, V, F)
            ohf.append(oh.reshape(128, VF))
        for c0, cn in CHUNKS:
            base = 2 * CWW + 2 * c0
            pk1[:, base:base + cn] = ohf[0][:, c0:c0 + cn]
            pk1[:, base + cn:base + 2 * cn] = ohf[1][:, c0:c0 + cn]

        ids_h = ids[b, LH * h:LH * (h + 1)]
        t = np.arange(LH)
        floc = ((129 * (t + LH * h)) >> 13) - 64 * h
        bf = np.zeros((KOUT, BFW), np.float32)
        bf[floc, DM + t] = 1.0
        bf[F + ids_h, DM + t] = 1.0
        bf[:, :DM] = rtimg
        in_maps.append({
            "pk1": pk1, "pk2": pk2, "nyw": nywr,
            "bfsel": bf.astype(bf16),
        })

    if _PROG is None:
        _PROG = _build_program()

    res = run_bass_kernel_spmd(_PROG, in_maps, core_ids=list(range(NCORES)))
    LAST_RESULT = res

    full = np.zeros((B, L, DM), np.float32)
    for core in range(NCORES):
        b, h = divmod(core, 2)
        full[b, LH * h:LH * (h + 1), :] = \
            res.results[core]["out"].astype(np.float32)
    return full


# revision 29
# speedup vs baseline: 1.0211x; 1.0211x over previous
"""Trainium2 Bass kernel for nn_CaduceusEmbeddingsSTFT.

out[b, t, :] = concat(emb_table[ids[b, t]],
                      proj(|STFT(onehot(ids[b]))| upsampled at frame f(t)))

Structure exploited:
  * nearest upsampling -> only 65 distinct STFT frame rows per core-half;
    the projection collapses to (65 x 2064) @ (2064 x 154).
  * STFT of one-hot signals: windowed frames are one-hot masks, so
    spec = onehot_frames @ (window * DFT) as matmuls (cos / sin).
  * every output row is concat(emb_row[id(t)], S[frame(t)]) -- built by a
    SINGLE matmul per 128-row tile: lhsT stacks the id one-hot (16 rows)
    and the frame one-hot (65 rows), rhs is the table [emb | S].

Precision: harness gate is rel_err < 2e-2; everything runs plain bf16
(fp32 PSUM accumulation), output DMA'd as bf16 and upcast on host.
Measured numpy sim of this scheme: rel err ~4.8e-3.

Sharding: 8 cores = 4 batches x 2 sequence halves; each core computes a
(4096, 512) output shard; boundary frame recomputed by both halves.

Perf design (per core): ~18 dummy matmuls pre-warm the PE HAM clock
while input DMAs land; DFT (6 streams x 1040 cols) -> |mag| on ACT/DVE
-> projection (16 K=128 MMs + 16 K=1 nyquist MMs, bias via DVE
broadcast-add into the rhs table) -> 32 fused N=512 output MMs; drains
alternate DVE/ACT; output leaves as bf16 (4 MB/core, ~12 us DMA).
"""

import numpy as np

V = 16
D_EMB = 358
D_STFT = 154
NFFT = 256
HOP = 64
NFREQ = 129
B, L = 4, 8192
LH = L // 2  # 4096 rows per core
F = 65  # frames per core (inclusive overlap frame)
VF = V * F  # 1040
DM = 512
NCORES = 8
NT = LH // 128  # 32 output tiles per core
NQ = NT // 4  # q-groups of 4 tiles
KOUT = V + F  # 81: stacked one-hot rows in the output matmul
CWW = 2 * NFREQ  # 258: per-c block width in cw (cos 0..127 | ny | sin 0..127)
# (start, size) chunks over the VF axis; multiples of F so projection
# lhsT slices [:, v*F:(v+1)*F] never cross a chunk boundary; <=512 f32
# per PSUM bank.
CHUNKS = [(0, 7 * F), (7 * F, 7 * F), (14 * F, 2 * F)]
NDUM = 34  # PE warm-up matmuls issued while input DMAs land
NFILL2 = 12  # keep-warm matmuls ready with chunk-2 |mag|
NFILL3 = 14  # keep-warm matmuls ready with chunk-3 |mag|

PK1W = 2 * VF + 2 * CWW  # [ohf0 | ohf1 | cw]
BFW = LH + DM  # [one-hot select cols | rhs-table init image]

_PROG = None
LAST_RESULT = None  # BassKernelResults of the most recent run (for harnesses)


def _build_program():
    import concourse.mybir as mybir
    import concourse.tile as tile
    from concourse import bacc

    f32 = mybir.dt.float32
    bf16 = mybir.dt.bfloat16
    AO = mybir.AluOpType
    AF = mybir.ActivationFunctionType

    nc = bacc.Bacc("TRN2", target_bir_lowering=False, debug=False,
                   num_devices=NCORES)

    pk1 = nc.dram_tensor("pk1", [128, PK1W], bf16, kind="ExternalInput")
    pk2 = nc.dram_tensor("pk2", [128, V * D_STFT], bf16, kind="ExternalInput")
    nyw = nc.dram_tensor("nyw", [1, V * D_STFT], bf16, kind="ExternalInput")
    bfsel = nc.dram_tensor("bfsel", [KOUT, BFW], bf16, kind="ExternalInput")
    out = nc.dram_tensor("out", [LH, DM], bf16, kind="ExternalOutput")

    with tile.TileContext(nc) as tc:
        with (
            tc.tile_pool(name="consts", bufs=1) as cpool,
            tc.tile_pool(name="work", bufs=1) as wpool,
            tc.tile_pool(name="tmp", bufs=2) as tpool,
            tc.tile_pool(name="ostg", bufs=3) as ospool,
        ):
            # ---- const loads (issue order = need order) ---------------------
            PK1 = cpool.tile([128, PK1W], bf16, tag="pk1")
            nc.sync.dma_start(out=PK1[:], in_=pk1[:])
            PK2 = cpool.tile([128, V * D_STFT], bf16, tag="pk2")
            nc.sync.dma_start(out=PK2[:], in_=pk2[:])
            BF = cpool.tile([KOUT, BFW], bf16, tag="bf")
            nc.sync.dma_start(out=BF[:], in_=bfsel[:])
            NYW = cpool.tile([1, V * D_STFT], bf16, tag="nyw")
            nc.sync.dma_start(out=NYW[:], in_=nyw[:])

            OHF = [PK1[:, 0:VF], PK1[:, VF:2 * VF]]
            CW = PK1[:, 2 * VF:]
            WP = PK2
            WNR = NYW

            # ---- on-chip work tiles ----------------------------------------
            ZW = wpool.tile([128, 128], bf16, tag="zw")
            nc.vector.memset(ZW[:], 0.0)
            # rhs table init: rows 0..64 [0 | bias], rows 65..80 [emb | 0]
            RT = wpool.tile([KOUT, DM], bf16, tag="rt")
            nc.vector.tensor_copy(out=RT[:], in_=BF[:, LH:])
            MAGH = wpool.tile([128, VF], bf16, tag="magh")
            NYB = wpool.tile([1, VF], bf16, tag="nyb")

            with tc.tile_pool(name="psum_s", bufs=1, space="PSUM") as psp:
                S = psp.tile([F, D_STFT], f32, tag="s")

                with (
                    tc.tile_pool(name="psum_dum", bufs=1, space="PSUM") as pdm,
                    tc.tile_pool(name="psum_re", bufs=2, space="PSUM") as pre,
                    tc.tile_pool(name="psum_im", bufs=2, space="PSUM") as pim,
                    tc.tile_pool(name="psum_ny", bufs=2, space="PSUM") as pny,
                ):
                    # PE warm-up: no input deps, scheduler runs these first;
                    # ~18 x (ldw+mm) ~= 3.4us busy -> HAM releases to 2.4 GHz
                    # right as the first real matmul's data lands.
                    DU = pdm.tile([128, 128], f32, tag="du")
                    for _ in range(NDUM):
                        nc.tensor.matmul(out=DU[:], lhsT=ZW[:], rhs=ZW[:],
                                         start=True, stop=True)

                    first_s = [True]

                    def proj_mm(lhsT, rhs, stop=False):
                        nc.tensor.matmul(out=S[:], lhsT=lhsT, rhs=rhs,
                                         start=first_s[0], stop=stop)
                        first_s[0] = False

                    for c0, cn in CHUNKS:
                        re = pre.tile([128, 7 * F], f32, tag="re")
                        im = pim.tile([128, 7 * F], f32, tag="im")
                        ny = pny.tile([1, 7 * F], f32, tag="ny")
                        for c in range(2):
                            cb = c * CWW
                            rhs = OHF[c][:, c0:c0 + cn]
                            nc.tensor.matmul(
                                out=re[:, :cn], lhsT=CW[:, cb:cb + 128],
                                rhs=rhs, start=(c == 0), stop=(c == 1))
                            nc.tensor.matmul(
                                out=im[:, :cn],
                                lhsT=CW[:, cb + NFREQ:cb + NFREQ + 128],
                                rhs=rhs, start=(c == 0), stop=(c == 1))
                            nc.tensor.matmul(
                                out=ny[:, :cn], lhsT=CW[:, cb + 128:cb + 129],
                                rhs=rhs, start=(c == 0), stop=(c == 1))
                        with tc.high_priority():
                            # |spec| = sqrt(re^2 + im^2); squares on ACT (DVE
                            # cannot read two PSUM operands)
                            t1 = tpool.tile([128, 7 * F], f32, tag="sq1")
                            t2 = tpool.tile([128, 7 * F], f32, tag="sq2")
                            nc.scalar.square(out=t1[:, :cn], in_=re[:, :cn])
                            nc.scalar.square(out=t2[:, :cn], in_=im[:, :cn])
                            nc.vector.tensor_tensor(
                                out=t1[:, :cn], in0=t1[:, :cn], in1=t2[:, :cn],
                                op=AO.add)
                            nc.scalar.sqrt(out=MAGH[:, c0:c0 + cn],
                                           in_=t1[:, :cn])
                            nc.scalar.activation(NYB[:, c0:c0 + cn],
                                                 ny[:, :cn], AF.Abs)
                            for v in range(c0 // F, (c0 + cn) // F):
                                proj_mm(MAGH[:, v * F:(v + 1) * F],
                                        WP[:, v * D_STFT:(v + 1) * D_STFT])
                            # nyquist bin: tiny K=1 matmuls off this chunk's
                            # |ny| slice -- distributed per chunk so the S
                            # tail after the last sqrt is only ~0.7us
                            for v in range(c0 // F, (c0 + cn) // F):
                                proj_mm(NYB[0:1, v * F:(v + 1) * F],
                                        WNR[0:1, v * D_STFT:(v + 1) * D_STFT],
                                        stop=(c0 + cn == VF
                                              and v == VF // F - 1))

                    # keep-warm fillers: become ready with chunk-2/3 |mag|,
                    # so the scheduler slots them into the PE idle window
                    # while the S tail runs on ACT/DVE. Without these the
                    # ~2.5us gap re-throttles HAM and the whole output phase
                    # runs at 1.2 GHz.
                    for _ in range(NFILL2):
                        nc.tensor.matmul(out=DU[:],
                                         lhsT=MAGH[:, 14 * F - 128:14 * F],
                                         rhs=ZW[:], start=True, stop=True)
                    for _ in range(NFILL3):
                        nc.tensor.matmul(out=DU[:],
                                         lhsT=MAGH[:, VF - 128:VF],
                                         rhs=ZW[:], start=True, stop=True)

                with tc.high_priority():
                    # rhs table rows 0..64 = S + bias image (in-place add)
                    nc.vector.tensor_tensor(
                        out=RT[0:F, D_EMB:DM], in0=S[:],
                        in1=RT[0:F, D_EMB:DM], op=AO.add)

            # ---- output: one fused matmul per 128-row tile ------------------
            # MMs (~216ns) outpace the DVE/ACT drains (~330ns/tile): the PE
            # micro-idles once PSUM fills, which re-throttles HAM mid-phase.
            # Two fillers per q-group (gated on this group's drains) keep
            # its activity window busy.
            with (
                tc.tile_pool(name="psum_out", bufs=7, space="PSUM") as pout,
                tc.tile_pool(name="psum_fil", bufs=1, space="PSUM") as pfil,
            ):
                for q in range(NQ):
                    os_ = ospool.tile([128, 4 * DM], bf16, tag="os")
                    for a in range(4):
                        ti = q * 4 + a
                        po = pout.tile([128, DM], f32, tag="po")
                        nc.tensor.matmul(
                            out=po[:], lhsT=BF[:, ti * 128:(ti + 1) * 128],
                            rhs=RT[0:KOUT, :], start=True, stop=True)
                        sl = os_[:, a * DM:(a + 1) * DM]
                        if a % 2 == 0:
                            nc.vector.tensor_copy(out=sl, in_=po[:])
                        else:
                            nc.scalar.copy(out=sl, in_=po[:])
                    DU2 = pfil.tile([128, 128], f32, tag="fil")
                    nc.tensor.matmul(out=DU2[:], lhsT=os_[:, 0:128],
                                     rhs=ZW[:], start=True, stop=True)
                    nc.tensor.matmul(out=DU2[:], lhsT=os_[:, DM:DM + 128],
                                     rhs=ZW[:], start=True, stop=True)
                    nc.sync.dma_start(
                        out=out[q * 512:(q + 1) * 512, :]
                        .rearrange("(a p) e -> p a e", p=128),
                        in_=os_[:].rearrange("p (a e) -> p a e", a=4))

    nc.finalize()
    return nc


def _host_consts():
    import ml_dtypes

    bf16 = ml_dtypes.bfloat16
    n = np.arange(NFFT)
    window = 0.5 - 0.5 * np.cos(2.0 * np.pi * n / NFFT)
    k = np.arange(NFREQ)
    ang = 2.0 * np.pi * np.outer(n, k) / NFFT  # (256, 129)
    wcos = (window[:, None] * np.cos(ang)).astype(np.float32)
    wsin = (window[:, None] * np.sin(ang)).astype(np.float32)
    cw = np.zeros((128, 2 * CWW), np.float32)
    for c in range(2):
        rows = slice(c * 128, (c + 1) * 128)
        blk = np.zeros((128, CWW), np.float32)
        blk[:, :128] = wcos[rows, :128]
        blk[:, 128] = wcos[rows][:, 128]  # nyquist cos column
        blk[:, NFREQ:NFREQ + 128] = wsin[rows, :128]
        cw[:, c * CWW:(c + 1) * CWW] = blk
    return cw.astype(bf16)


def kernel(input_ids, emb_table, proj_w, proj_b):
    global _PROG, LAST_RESULT
    import ml_dtypes

    from concourse.bass_utils import run_bass_kernel_spmd

    bf16 = ml_dtypes.bfloat16
    ids = np.asarray(input_ids).astype(np.int64)
    emb = np.asarray(emb_table).astype(np.float32)
    pw = np.asarray(proj_w).astype(np.float32)
    pb = np.asarray(proj_b).astype(np.float32)

    cw = _host_consts()

    # pk1 cols [2*VF:] = cw; per-core ohf fills cols [:2*VF]
    # pk2: proj weights, rows k=0..127, cols v*154+o  (proj_w row i=k*V+v)
    pk2 = np.zeros((128, V * D_STFT), np.float32)
    for v in range(V):
        pk2[:, v * D_STFT:(v + 1) * D_STFT] = pw[np.arange(128) * V + v]
    pk2 = pk2.astype(bf16)

    # nyw: nyquist-bin proj weights as a single partition-0 row
    nywr = np.zeros((1, V * D_STFT), np.float32)
    for v in range(V):
        nywr[0, v * D_STFT:(v + 1) * D_STFT] = pw[128 * V + v]
    nywr = nywr.astype(bf16)

    # rhs-table init image: rows 0..64 [0 | bias], rows 65..80 [emb | 0]
    rtimg = np.zeros((KOUT, DM), np.float32)
    rtimg[:F, D_EMB:] = pb[None, :]
    rtimg[F:, :D_EMB] = emb

    vr = np.arange(V)
    in_maps = []
    for core in range(NCORES):
        b, h = divmod(core, 2)
        padded = np.pad(ids[b], 128, mode="reflect")
        seg = padded[LH * h:LH * h + 64 * (F - 1) + NFFT]  # (4352,)
        pk1 = np.zeros((128, PK1W), bf16)
        for c in range(2):
            sv = seg[(128 * c + np.arange(128))[:, None]
                     + 64 * np.arange(F)[None, :]]  # (128, F)
            oh = (sv[:, None, :] == vr[None, :, None])  # (128, V, F)
            pk1[:, c * VF:(c + 1) * VF] = oh.reshape(128, VF)
        pk1[:, 2 * VF:] = cw

        ids_h = ids[b, LH * h:LH * (h + 1)]
        t = np.arange(LH)
        floc = ((129 * (t + LH * h)) >> 13) - 64 * h
        bf = np.zeros((KOUT, BFW), np.float32)
        bf[floc, t] = 1.0
        bf[F + ids_h, t] = 1.0
        bf[:, LH:] = rtimg
        in_maps.append({
            "pk1": pk1, "pk2": pk2, "nyw": nywr,
            "bfsel": bf.astype(bf16),
        })

    if _PROG is None:
        _PROG = _build_program()

    res = run_bass_kernel_spmd(_PROG, in_maps, core_ids=list(range(NCORES)))
    LAST_RESULT = res

    full = np.zeros((B, L, DM), np.float32)
    for core in range(NCORES):
        b, h = divmod(core, 2)
        full[b, LH * h:LH * (h + 1), :] = \
            res.results[core]["out"].astype(np.float32)
    return full
